# revision 7
# baseline (speedup 1.0000x reference)
"""Single-head attention (B=2, S=2048, D=2048, fp32) on 8 trn2 NeuronCores.

Sharding: sequence-parallel. The 4096 tokens (B*S) are split 512/core; cores
0-3 hold batch 0, cores 4-7 batch 1. Each core computes Q^T, K^T, V for its
512 tokens, K^T/V shards are all-gathered within each 4-core group (one group
per batch), then each core computes scores -> softmax -> attn@V -> @W_o for
its 512 queries against the full 2048 keys of its batch.

All matmuls run as fp32r (full PE rate at N=512, ~tf32 precision; inputs and
all producer chains typed float32r to satisfy the BIR verifier).

per-core phases (each 256 matmuls of K=128, M=128, N=512):
  B: KT_shard(e,t)  = mm(lhsT=W_k[d,e], rhs=xt[d,t])      -> DRAM, AllGather
  C: V_shard(t,e)   = mm(lhsT=xt[d,t],  rhs=W_v[d,e])     -> DRAM, AllGather
  D: QT(e,q)        = mm(lhsT=W_q'[d,e], rhs=xt[d,q])     -> SBUF  (W_q'=W_q/sqrt(D))
  E: scores(q,k)    = mm(lhsT=QT[e,q], rhs=KTg[e,k]); exp (no max-sub; |s|<~8)
     with accum_out row-sums; attnT via PE transposes
  F: outT(e,q)      = mm(lhsT=Vg[k,e], rhs=attnT[k,q])
  G: final(q,d)     = mm(lhsT=outT[e,q], rhs=W_o[e,d]) * (1/rowsum) -> out DRAM

All DMA loads are (128, 512) chunks (2KB per partition line).
"""
import math
import numpy as np

import concourse.bass as bass
import concourse.mybir as mybir
import concourse.tile as tile
from concourse import bacc
from concourse.bass_utils import run_bass_kernel_spmd
from concourse.masks import make_identity

F32 = mybir.dt.float32
F32R = mybir.dt.float32r

D = 2048          # d_model
B = 2
S = 2048
NCORES = 8
GS = 4            # group size (cores per batch)
TOK = 512         # tokens per core
P = 128
NT = D // P       # 16 tiles along d/e
QT_N = TOK // P   # 4 q tiles per core
KC_N = S // 512   # 4 key chunks of 512


def build_attn(n_iters=1, with_collective=True, psum_bufs=6, w_bufs=4, kv_bufs=4):
    """Build the SPMD attention kernel. n_iters>1 wraps ALL compute phases in
    a timing loop (collectives must be disabled for that)."""
    assert n_iters == 1 or with_collective is False
    nc = bacc.Bacc("TRN2", target_bir_lowering=False, debug=False, num_devices=NCORES)

    xt = nc.dram_tensor("xt", [D, TOK], F32R, kind="ExternalInput")
    wq = nc.dram_tensor("wq", [D, D], F32R, kind="ExternalInput")
    wk = nc.dram_tensor("wk", [D, D], F32R, kind="ExternalInput")
    wv = nc.dram_tensor("wv", [D, D], F32R, kind="ExternalInput")
    wo = nc.dram_tensor("wo", [D, D], F32R, kind="ExternalInput")
    out = nc.dram_tensor("out", [TOK, D], F32, kind="ExternalOutput")

    with tile.TileContext(nc) as tc:
        with (
            tc.tile_pool(name="dram", bufs=1, space="DRAM") as dram,
            tc.tile_pool(name="big", bufs=1) as big,
            tc.tile_pool(name="wpool", bufs=w_bufs) as wpool,
            tc.tile_pool(name="kvpool", bufs=kv_bufs) as kvpool,
            tc.tile_pool(name="evpool", bufs=3) as evpool,
            tc.tile_pool(name="misc", bufs=1) as misc,
            tc.tile_pool(name="ps", bufs=psum_bufs, space="PSUM") as ps,
            tc.tile_pool(name="pst", bufs=2, space="PSUM") as pst,
        ):
            kt_shard = dram.tile([D, TOK], F32R)
            v_shard = dram.tile([TOK, D], F32R)
            kt_g = dram.tile([GS * D, TOK], F32R)    # [s*D + e, k_local]
            v_g = dram.tile([GS * TOK, D], F32R)     # [k, e]

            xt_sb = big.tile([P, NT, TOK], F32R)
            qt_sb = big.tile([P, NT, TOK], F32R)
            attn_sb = big.tile([P, QT_N, S], F32)
            attnT_sb = big.tile([P, NT, TOK], F32R)
            outT_sb = big.tile([P, NT, TOK], F32R)

            ident = misc.tile([P, P], F32)
            make_identity(nc, ident)
            sums = misc.tile([P, QT_N, KC_N], F32)
            recip = misc.tile([P, QT_N], F32)
            tot = misc.tile([P, QT_N], F32)

            # ---- load x^T into SBUF
            for dt in range(NT):
                nc.sync.dma_start(xt_sb[:, dt, :], xt[dt * P:(dt + 1) * P, :])

            def proj_to_T(w_dram, dest_cb, pfx):
                """QT/KT-style projection: out[e,t] accumulated over d.
                Weight loads are (128,512) chunks covering 4 e-tiles."""
                for eg in range(NT // 4):          # groups of 4 e-tiles
                    psums = [ps.tile([P, 512], F32, tag="mm", name=f"{pfx}{i}")
                             for i in range(4)]
                    for dt in range(NT):
                        w_t = wpool.tile([P, 512], F32R, tag="w512", name=f"{pfx}w")
                        nc.sync.dma_start(
                            w_t[:], w_dram[dt * P:(dt + 1) * P,
                                           eg * 512:(eg + 1) * 512])
                        for j in range(4):
                            nc.tensor.matmul(
                                psums[j][:], w_t[:, j * P:(j + 1) * P],
                                xt_sb[:, dt, :],
                                start=(dt == 0), stop=(dt == NT - 1))
                    for j in range(4):
                        dest_cb(eg * 4 + j, psums[j])

            def b_dest(et, psum):
                ev = evpool.tile([P, 512], F32R, tag="ev", name="evb")
                nc.scalar.copy(ev[:], psum[:])
                nc.sync.dma_start(kt_shard[et * P:(et + 1) * P, :], ev[:])

            def phase_c():
                for ec in range(4):
                    psums = [ps.tile([P, 512], F32, tag="mm", name=f"pv{i}")
                             for i in range(QT_N)]
                    for dt in range(NT):
                        wv_t = wpool.tile([P, 512], F32R, tag="w512", name="wvt")
                        nc.sync.dma_start(
                            wv_t[:], wv[dt * P:(dt + 1) * P, ec * 512:(ec + 1) * 512])
                        for tt in range(QT_N):
                            nc.tensor.matmul(
                                psums[tt][:], xt_sb[:, dt, tt * P:(tt + 1) * P],
                                wv_t[:],
                                start=(dt == 0), stop=(dt == NT - 1))
                    for tt in range(QT_N):
                        ev = evpool.tile([P, 512], F32R, tag="ev", name="evc")
                        nc.scalar.copy(ev[:], psums[tt][:])
                        nc.sync.dma_start(
                            v_shard[tt * P:(tt + 1) * P, ec * 512:(ec + 1) * 512], ev[:])

            def phases_defg():
                # ---- phase D: QT
                def d_dest(et, psum):
                    nc.scalar.copy(qt_sb[:, et, :], psum[:])
                proj_to_T(wq, d_dest, "pd")

                # ---- phase E: scores + exp + transpose
                for kc in range(KC_N):
                    psums = [ps.tile([P, 512], F32, tag="mm", name=f"pe{i}")
                             for i in range(QT_N)]
                    for et in range(NT):
                        kt_t = kvpool.tile([P, 512], F32R, tag="kt")
                        nc.sync.dma_start(
                            kt_t[:], kt_g[kc * D + et * P: kc * D + (et + 1) * P, :])
                        for qt in range(QT_N):
                            nc.tensor.matmul(
                                psums[qt][:],
                                qt_sb[:, et, qt * P:(qt + 1) * P],
                                kt_t[:],
                                start=(et == 0), stop=(et == NT - 1))
                    for qt in range(QT_N):
                        nc.scalar.activation(
                            attn_sb[:, qt, kc * 512:(kc + 1) * 512], psums[qt][:],
                            mybir.ActivationFunctionType.Exp,
                            accum_out=sums[:, qt, kc:kc + 1])
                # row sums + reciprocal
                for qt in range(QT_N):
                    nc.vector.reduce_sum(tot[:, qt:qt + 1], sums[:, qt, :],
                                         axis=mybir.AxisListType.X)
                nc.vector.reciprocal(recip[:], tot[:])

                # transposes attn (q,k) -> attnT (k,q)
                for kt in range(NT):
                    for qt in range(QT_N):
                        pt = pst.tile([P, P], F32, tag="tp")
                        nc.tensor.transpose(
                            pt[:], attn_sb[:, qt, kt * P:(kt + 1) * P], ident[:])
                        nc.scalar.copy(attnT_sb[:, kt, qt * P:(qt + 1) * P], pt[:])

                # ---- phase F: outT (v loads are (128,512) row chunks = 4 e-tiles)
                for eg in range(NT // 4):
                    psums = [ps.tile([P, 512], F32, tag="mm", name=f"pf{i}")
                             for i in range(4)]
                    for kt in range(NT):
                        v_t = kvpool.tile([P, 512], F32R, tag="vt")
                        nc.sync.dma_start(
                            v_t[:], v_g[kt * P:(kt + 1) * P, eg * 512:(eg + 1) * 512])
                        for j in range(4):
                            nc.tensor.matmul(
                                psums[j][:], v_t[:, j * P:(j + 1) * P],
                                attnT_sb[:, kt, :],
                                start=(kt == 0), stop=(kt == NT - 1))
                    for j in range(4):
                        nc.scalar.copy(outT_sb[:, eg * 4 + j, :], psums[j][:])

                # ---- phase G: final
                for dc in range(4):
                    psums = [ps.tile([P, 512], F32, tag="mm", name=f"pg{i}")
                             for i in range(QT_N)]
                    for et in range(NT):
                        wo_t = wpool.tile([P, 512], F32R, tag="w512", name="wot")
                        nc.sync.dma_start(
                            wo_t[:], wo[et * P:(et + 1) * P, dc * 512:(dc + 1) * 512])
                        for qt in range(QT_N):
                            nc.tensor.matmul(
                                psums[qt][:],
                                outT_sb[:, et, qt * P:(qt + 1) * P],
                                wo_t[:],
                                start=(et == 0), stop=(et == NT - 1))
                    for qt in range(QT_N):
                        evf = evpool.tile([P, 512], F32, tag="evf")
                        nc.vector.tensor_scalar_mul(evf[:], psums[qt][:],
                                                    recip[:, qt:qt + 1])
                        nc.sync.dma_start(
                            out[qt * P:(qt + 1) * P, dc * 512:(dc + 1) * 512], evf[:])

            def whole_body():
                proj_to_T(wk, b_dest, "pb")
                if with_collective in (True, "k"):
                    nc.gpsimd.collective_compute(
                        "AllGather", mybir.AluOpType.bypass,
                        replica_groups=[[0, 1, 2, 3], [4, 5, 6, 7]],
                        ins=[kt_shard[:].opt()], outs=[kt_g[:].opt()],
                    )
                phase_c()
                if with_collective in (True, "v"):
                    nc.gpsimd.collective_compute(
                        "AllGather", mybir.AluOpType.bypass,
                        replica_groups=[[0, 1, 2, 3], [4, 5, 6, 7]],
                        ins=[v_shard[:].opt()], outs=[v_g[:].opt()],
                    )
                phases_defg()

            if n_iters == 1:
                whole_body()
            else:
                with tc.For_i(0, n_iters, 1):
                    whole_body()

    nc.compile()
    return nc


_CACHED = {}


def _get_nc():
    if "nc" not in _CACHED:
        _CACHED["nc"] = build_attn()
    return _CACHED["nc"]


def _make_in_maps(inputs):
    x = np.asarray(inputs["x"], np.float32)
    W_q = np.asarray(inputs["W_q"], np.float32)
    W_k = np.asarray(inputs["W_k"], np.float32)
    W_v = np.asarray(inputs["W_v"], np.float32)
    W_o = np.asarray(inputs["W_o"], np.float32)

    scale = np.float32(1.0 / math.sqrt(D))
    wq_s = np.ascontiguousarray(W_q * scale)
    wk_c = np.ascontiguousarray(W_k)
    wv_c = np.ascontiguousarray(W_v)
    wo_c = np.ascontiguousarray(W_o)

    toks = x.reshape(B * S, D)              # (4096, 2048)
    xt_full = np.ascontiguousarray(toks.T)  # (2048, 4096)

    in_maps = []
    for c in range(NCORES):
        in_maps.append({
            "xt": np.ascontiguousarray(xt_full[:, c * TOK:(c + 1) * TOK]),
            "wq": wq_s, "wk": wk_c, "wv": wv_c, "wo": wo_c,
        })
    return in_maps


def kernel(x, W_q, W_k, W_v, W_o):
    in_maps = _make_in_maps(dict(x=x, W_q=W_q, W_k=W_k, W_v=W_v, W_o=W_o))
    nc = _get_nc()
    res = run_bass_kernel_spmd(nc, in_maps, core_ids=list(range(NCORES)))
    rows = np.concatenate([res.results[c]["out"] for c in range(NCORES)], axis=0)
    return rows.reshape(B, S, D)


# revision 17
# speedup vs baseline: 5.4234x; 5.4234x over previous
"""Single-head attention (B=2, S=2048, D=2048, fp32) on 8 trn2 NeuronCores.

Sharding: sequence-parallel. The 4096 tokens (B*S) are split 512/core; cores
0-3 hold batch 0, cores 4-7 batch 1. Each core computes Q^T, K^T, V for its
512 tokens, K^T/V shards are all-gathered within each 4-core group (one group
per batch), then each core computes scores -> softmax -> attn@V -> @W_o for
its 512 queries against the full 2048 keys of its batch.

All matmuls run as fp32r (full PE rate at N=512, ~tf32 precision; inputs and
all producer chains typed float32r to satisfy the BIR verifier).

per-core phases (each 256 matmuls of K=128, M=128, N=512):
  B: KT_shard(e,t)  = mm(lhsT=W_k[d,e], rhs=xt[d,t])      -> DRAM, AllGather
  C: V_shard(t,e)   = mm(lhsT=xt[d,t],  rhs=W_v[d,e])     -> DRAM, AllGather
  D: QT(e,q)        = mm(lhsT=W_q'[d,e], rhs=xt[d,q])     -> SBUF  (W_q'=W_q/sqrt(D))
  E: scores(q,k)    = mm(lhsT=QT[e,q], rhs=KTg[e,k]); exp (no max-sub; |s|<~8)
     with accum_out row-sums; attnT via PE transposes
  F: outT(e,q)      = mm(lhsT=Vg[k,e], rhs=attnT[k,q])
  G: final(q,d)     = mm(lhsT=outT[e,q], rhs=W_o[e,d]) * (1/rowsum) -> out DRAM

All DMA loads are (128, 512) chunks (2KB per partition line).
"""
import math
import numpy as np

import concourse.bass as bass
import concourse.mybir as mybir
import concourse.tile as tile
from concourse import bacc
from concourse.bass_utils import run_bass_kernel_spmd
from concourse.masks import make_identity

F32 = mybir.dt.float32
F32R = mybir.dt.float32r

D = 2048          # d_model
B = 2
S = 2048
NCORES = 8
GS = 4            # group size (cores per batch)
TOK = 512         # tokens per core
P = 128
NT = D // P       # 16 tiles along d/e
QT_N = TOK // P   # 4 q tiles per core
KC_N = S // 512   # 4 key chunks of 512


def build_attn(n_iters=1, with_collective=True, psum_bufs=8, w_bufs=24, kv_bufs=8, skip_w_reload=False):
    """Build the SPMD attention kernel. n_iters>1 wraps ALL compute phases in
    a timing loop (collectives must be disabled for that)."""
    assert n_iters == 1 or with_collective is False
    nc = bacc.Bacc("TRN2", target_bir_lowering=False, debug=False, num_devices=NCORES)

    xt = nc.dram_tensor("xt", [D, TOK], F32R, kind="ExternalInput")
    wq = nc.dram_tensor("wq", [D, D], F32R, kind="ExternalInput")
    wk = nc.dram_tensor("wk", [D, D], F32R, kind="ExternalInput")
    wv = nc.dram_tensor("wv", [D, D], F32R, kind="ExternalInput")
    wo = nc.dram_tensor("wo", [D, D], F32R, kind="ExternalInput")
    out = nc.dram_tensor("out", [TOK, D], F32, kind="ExternalOutput")

    with tile.TileContext(nc) as tc:
        with (
            tc.tile_pool(name="dram", bufs=1, space="DRAM") as dram,
            tc.tile_pool(name="big", bufs=1) as big,
            tc.tile_pool(name="stream", bufs=w_bufs) as stream,
            tc.tile_pool(name="qtpool", bufs=NT) as qtpool,
            tc.tile_pool(name="evpool", bufs=3) as evpool,
            tc.tile_pool(name="attnpool", bufs=4) as attnpool,
            tc.tile_pool(name="misc", bufs=1) as misc,
            tc.tile_pool(name="ps", bufs=psum_bufs, space="PSUM") as ps,
        ):
            kt_shard = dram.tile([D, TOK], F32R)
            v_shard = dram.tile([TOK, D], F32R)
            kt_g = dram.tile([GS * D, TOK], F32R)    # [s*D + e, k_local]
            v_g = dram.tile([GS * TOK, D], F32R)     # [k, e]

            xt_sb = [big.tile([P, TOK], F32R, name=f"xt{i}") for i in range(NT)]

            attnT_sb = [big.tile([P, TOK], F32R, name=f"attnT{i}") for i in range(NT)]


            ident = misc.tile([P, P], F32)
            make_identity(nc, ident)
            sums = misc.tile([P, QT_N, KC_N], F32)
            recip = misc.tile([P, QT_N], F32)
            tot = misc.tile([P, QT_N], F32)

            _eng_i = [0]
            _engines = [nc.sync, nc.scalar]

            def LD(dst, src_ap):
                e = _engines[_eng_i[0] % len(_engines)]
                _eng_i[0] += 1
                e.dma_start(dst, src_ap)

            # ---- load x^T into SBUF
            for dt in range(NT):
                nc.sync.dma_start(xt_sb[dt][:], xt[dt * P:(dt + 1) * P, :])

            def stream_group(pfx, load_fn):
                """Load 16 (128,512) chunks via round-robin engines."""
                ts = []
                for i in range(NT):
                    t = stream.tile([P, 512], F32R, tag="stream", name=f"{pfx}{i}")
                    if i == 0 or not skip_w_reload:
                        LD(t[:], load_fn(i))
                        ts.append(t)
                    else:
                        ts.append(ts[0])
                return ts

            def proj_to_T(w_dram, dest_cb, pfx):
                """QT/KT-style projection: out[e,t] = sum_d W[d,e]*xt[d,t].
                16-deep same-bank accumulation chains (chain16 pattern)."""
                for eg in range(NT // 4):
                    wts = stream_group(pfx, lambda dt: w_dram[
                        dt * P:(dt + 1) * P, eg * 512:(eg + 1) * 512])
                    psums = [ps.tile([P, 512], F32, tag="mm", name=f"{pfx}p{i}")
                             for i in range(4)]
                    for half in range(2):
                        for j in range(4):
                            for dt8 in range(8):
                                dt = half * 8 + dt8
                                nc.tensor.matmul(
                                    psums[j][:], wts[dt][:, j * P:(j + 1) * P],
                                    xt_sb[dt][:],
                                    start=(dt == 0), stop=(dt == NT - 1))
                    for j in range(4):
                        dest_cb(eg * 4 + j, psums[j])

            def b_dest(et, psum):
                ev = evpool.tile([P, 512], F32R, tag="ev", name="evb")
                nc.scalar.copy(ev[:], psum[:])
                nc.sync.dma_start(kt_shard[et * P:(et + 1) * P, :], ev[:])

            def phase_c():
                for ec in range(4):
                    wvs = stream_group("cw", lambda dt: wv[
                        dt * P:(dt + 1) * P, ec * 512:(ec + 1) * 512])
                    psums = [ps.tile([P, 512], F32, tag="mm", name=f"pvp{i}")
                             for i in range(4)]
                    for half in range(2):
                        for tt in range(QT_N):
                            for dt8 in range(8):
                                dt = half * 8 + dt8
                                nc.tensor.matmul(
                                    psums[tt][:], xt_sb[dt][:, tt * P:(tt + 1) * P],
                                    wvs[dt][:],
                                    start=(dt == 0), stop=(dt == NT - 1))
                    for tt in range(QT_N):
                        ev = evpool.tile([P, 512], F32R, tag="ev", name="evc")
                        nc.scalar.copy(ev[:], psums[tt][:])
                        nc.sync.dma_start(
                            v_shard[tt * P:(tt + 1) * P, ec * 512:(ec + 1) * 512], ev[:])

            def phases_defg():
                # ---- phase D: QT (tiles share slots with outT via tag)
                qt_sb = [qtpool.tile([P, TOK], F32R, tag="qo", name=f"qt{i}")
                         for i in range(NT)]

                def d_dest(et, psum):
                    nc.scalar.copy(qt_sb[et][:], psum[:])
                proj_to_T(wq, d_dest, "pd")

                # ---- phase E: scores + exp + inline transposes
                for kc in range(KC_N):
                    kts = stream_group("ek", lambda et: kt_g[
                        kc * D + et * P: kc * D + (et + 1) * P, :])
                    psums = [ps.tile([P, 512], F32, tag="mm", name=f"pep{i}")
                             for i in range(4)]
                    for half in range(2):
                        for qt in range(QT_N):
                            for et8 in range(8):
                                et = half * 8 + et8
                                nc.tensor.matmul(
                                    psums[qt][:],
                                    qt_sb[et][:, qt * P:(qt + 1) * P],
                                    kts[et][:],
                                    start=(et == 0), stop=(et == NT - 1))
                    for qt in range(QT_N):
                        ax = attnpool.tile([P, 512], F32, tag="ax", name="ax")
                        nc.scalar.activation(
                            ax[:], psums[qt][:],
                            mybir.ActivationFunctionType.Exp,
                            accum_out=sums[:, qt, kc:kc + 1])
                        for j in range(4):
                            pt = ps.tile([P, P], F32, tag="mm", name="pt")
                            nc.tensor.transpose(
                                pt[:], ax[:, j * P:(j + 1) * P], ident[:])
                            nc.scalar.copy(
                                attnT_sb[kc * 4 + j][:, qt * P:(qt + 1) * P], pt[:])
                # row sums + reciprocal
                for qt in range(QT_N):
                    nc.vector.reduce_sum(tot[:, qt:qt + 1], sums[:, qt, :],
                                         axis=mybir.AxisListType.X)
                nc.vector.reciprocal(recip[:], tot[:])

                # ---- phase F: outT (slots freed by qt after phase E)
                outT_sb = [qtpool.tile([P, TOK], F32R, tag="qo", name=f"outT{i}")
                           for i in range(NT)]
                for eg in range(NT // 4):
                    vts = stream_group("fv", lambda kt: v_g[
                        kt * P:(kt + 1) * P, eg * 512:(eg + 1) * 512])
                    psums = [ps.tile([P, 512], F32, tag="mm", name=f"pfp{i}")
                             for i in range(4)]
                    for half in range(2):
                        for j in range(4):
                            for kt8 in range(8):
                                kt = half * 8 + kt8
                                nc.tensor.matmul(
                                    psums[j][:], vts[kt][:, j * P:(j + 1) * P],
                                    attnT_sb[kt][:],
                                    start=(kt == 0), stop=(kt == NT - 1))
                    for j in range(4):
                        nc.scalar.copy(outT_sb[eg * 4 + j][:], psums[j][:])

                # ---- phase G: final
                for dc in range(4):
                    wos = stream_group("gw", lambda et: wo[
                        et * P:(et + 1) * P, dc * 512:(dc + 1) * 512])
                    psums = [ps.tile([P, 512], F32, tag="mm", name=f"pgp{i}")
                             for i in range(4)]
                    for half in range(2):
                        for qt in range(QT_N):
                            for et8 in range(8):
                                et = half * 8 + et8
                                nc.tensor.matmul(
                                    psums[qt][:],
                                    outT_sb[et][:, qt * P:(qt + 1) * P],
                                    wos[et][:],
                                    start=(et == 0), stop=(et == NT - 1))
                    for qt in range(QT_N):
                        evf = evpool.tile([P, 512], F32, tag="evf")
                        nc.vector.tensor_scalar_mul(evf[:], psums[qt][:],
                                                    recip[:, qt:qt + 1])
                        nc.sync.dma_start(
                            out[qt * P:(qt + 1) * P, dc * 512:(dc + 1) * 512], evf[:])

            def whole_body():
                proj_to_T(wk, b_dest, "pb")
                if with_collective in (True, "k"):
                    nc.gpsimd.collective_compute(
                        "AllGather", mybir.AluOpType.bypass,
                        replica_groups=[[0, 1, 2, 3], [4, 5, 6, 7]],
                        ins=[kt_shard[:].opt()], outs=[kt_g[:].opt()],
                    )
                phase_c()
                if with_collective in (True, "v"):
                    nc.gpsimd.collective_compute(
                        "AllGather", mybir.AluOpType.bypass,
                        replica_groups=[[0, 1, 2, 3], [4, 5, 6, 7]],
                        ins=[v_shard[:].opt()], outs=[v_g[:].opt()],
                    )
                phases_defg()

            if n_iters == 1:
                whole_body()
            else:
                with tc.For_i(0, n_iters, 1):
                    whole_body()

    nc.compile()
    return nc


_CACHED = {}


def _get_nc():
    if "nc" not in _CACHED:
        _CACHED["nc"] = build_attn()
    return _CACHED["nc"]


def _make_in_maps(inputs):
    x = np.asarray(inputs["x"], np.float32)
    W_q = np.asarray(inputs["W_q"], np.float32)
    W_k = np.asarray(inputs["W_k"], np.float32)
    W_v = np.asarray(inputs["W_v"], np.float32)
    W_o = np.asarray(inputs["W_o"], np.float32)

    scale = np.float32(1.0 / math.sqrt(D))
    wq_s = np.ascontiguousarray(W_q * scale)
    wk_c = np.ascontiguousarray(W_k)
    wv_c = np.ascontiguousarray(W_v)
    wo_c = np.ascontiguousarray(W_o)

    toks = x.reshape(B * S, D)              # (4096, 2048)
    xt_full = np.ascontiguousarray(toks.T)  # (2048, 4096)

    in_maps = []
    for c in range(NCORES):
        in_maps.append({
            "xt": np.ascontiguousarray(xt_full[:, c * TOK:(c + 1) * TOK]),
            "wq": wq_s, "wk": wk_c, "wv": wv_c, "wo": wo_c,
        })
    return in_maps


def kernel(x, W_q, W_k, W_v, W_o):
    in_maps = _make_in_maps(dict(x=x, W_q=W_q, W_k=W_k, W_v=W_v, W_o=W_o))
    nc = _get_nc()
    res = run_bass_kernel_spmd(nc, in_maps, core_ids=list(range(NCORES)))
    rows = np.concatenate([res.results[c]["out"] for c in range(NCORES)], axis=0)
    return rows.reshape(B, S, D)
